# revision 37
# baseline (speedup 1.0000x reference)
"""Trainium2 Bass kernel for a 2-layer custom GRU (batch=256, seq=512,
d_in=256, d_h=512), data-parallel over batch across 8 NeuronCores.

Strategy
--------
- combined = [h, x]; the reference only uses gate chunks z, r (cols 0:1024
  of Wz) -> the third chunk of Wz is dead weight and is trimmed.
- Per layer, weights split into h-part (rows 0:512) and x-part (rows 512:).
  x-contributions (incl. bias) are precomputed chunk-wise as dense matmuls
  (layer 0 from x, layer 1 from layer-0's output sequence), so the serial
  recurrence only does K=512 matmuls per gate.
- Recurrent state h is kept transposed (dims on partitions, batch on free)
  so it is directly the stationary lhsT of the gate matmuls and elementwise
  ops run on full 128 partitions. Gate matmul outputs (batch x dims in
  PSUM) are copied to SBUF and PE-transposed back.
- All matmuls run in float32r (fp32 storage, 1 cycle/column at N>=256).
- Chunks of 8 timesteps pipeline: layer-1 recurrence of chunk c overlaps
  layer-0 recurrence of chunk c+1 (independent dependency chains).
- PSUM tiles are STATIC (one allocation, no pool rotation): the walrus
  encoding of the self-loading f32/f32r matmul holds only ONE semaphore
  wait, so matmuls cannot carry pool-slot release waits on top of their
  producer wait. With static tiles + the emission order below, every PE
  instruction needs at most one wait (WAR hazards are transitively covered
  through same-engine semaphore ticks).
"""

import os
import sys

for _p in ("/opt/trn_rl_repo", "/root/.axon_site/_ro/trn_rl_repo"):
    if os.path.isdir(_p) and _p not in sys.path:
        sys.path.insert(0, _p)

import numpy as np

import concourse.bass as bass
import concourse.mybir as mybir
import concourse.tile as tile
from concourse.alu_op_type import AluOpType
from concourse.bass_utils import run_bass_kernel_spmd
from concourse.tile import add_dep_helper

F32 = mybir.dt.float32
F32R = mybir.dt.float32r
AF = mybir.ActivationFunctionType

B, S, DIN, DH = 256, 512, 256, 512
NCORES = 8
BL = B // NCORES          # 32 batch rows per core
CH = 8                    # timesteps per chunk
NB = CH * BL              # 256 step-batch columns per chunk
NZR = 2 * DH              # 1024: trimmed z|r gate width
NOUT = 3 * DH             # 1536: z|r|c combined output width
KH = DH // 128            # 4 k-tiles for the h contraction
MT = NOUT // 128          # 12 m-tiles of precomputed contributions


class Ctx:
    pass


def _recurrence_step(cx, lay, h, s, h0ch=None):
    """One GRU cell step. h: (128, KH, BL) f32r transposed state.
    lay: per-layer statics (wzr, wc, pzc, pr, pt, ca). Returns new state."""
    nc = cx.nc
    sb = cx.sb
    ident = cx.ident
    wzr, wc, pzc, pr, pt, ca = lay.wzr, lay.wc, lay.pzc, lay.pr, lay.pt, lay.ca
    cb = slice(BL * s, BL * s + BL)

    # gate pre-activations: r first (its WAR+RAW deps are both DVE ticks)
    for k in range(KH):
        nc.tensor.matmul(pr, h[:, k], wzr[:, k, DH:NZR], start=(k == 0), stop=(k == KH - 1))
    for k in range(KH):
        nc.tensor.matmul(pzc, h[:, k], wzr[:, k, 0:DH], start=(k == 0), stop=(k == KH - 1))

    # PSUM -> SBUF (rc on DVE so the r-transposes' deps stay on one engine)
    rc = sb.tile([BL, DH], F32, tag="rc%d" % lay.li)
    zc = sb.tile([BL, DH], F32, tag="zc%d" % lay.li)
    nc.vector.tensor_copy(rc, pr)
    nc.scalar.activation(zc, pzc, AF.Copy)
    for j in range(KH):
        nc.tensor.transpose(pt[:, KH + j], rc[:, 128 * j : 128 * (j + 1)], ident[:BL, :BL])
    for j in range(KH):
        nc.tensor.transpose(pt[:, j], zc[:, 128 * j : 128 * (j + 1)], ident[:BL, :BL])

    # zr = sigmoid(pre + x_contrib)
    zrt = sb.tile([128, 2 * KH, BL], F32, tag="zrt%d" % lay.li)
    nc.vector.tensor_tensor(zrt, pt[:, 0 : 2 * KH], ca[:, 0 : 2 * KH, cb], AluOpType.add)
    nc.scalar.activation(zrt, zrt, AF.Sigmoid)

    # candidate: (r*h) @ Wc_h, reusing the z bank (WAR on zc copy is covered)
    rh = sb.tile([128, KH, BL], F32R, tag="rh%d" % lay.li)
    nc.vector.tensor_tensor(rh, zrt[:, KH : 2 * KH], h, AluOpType.mult)
    for k in range(KH):
        nc.tensor.matmul(pzc, rh[:, k], wc[:, k], start=(k == 0), stop=(k == KH - 1))
    cc = sb.tile([BL, DH], F32, tag="cc%d" % lay.li)
    nc.scalar.activation(cc, pzc, AF.Copy)
    for j in range(KH):
        nc.tensor.transpose(pt[:, 2 * KH + j], cc[:, 128 * j : 128 * (j + 1)], ident[:BL, :BL])

    # h_tilde = tanh(pre + x_contrib); h' = h + z * (h_tilde - h)
    cht = sb.tile([128, KH, BL], F32, tag="cht%d" % lay.li)
    nc.vector.tensor_tensor(
        cht, pt[:, 2 * KH : 3 * KH], ca[:, 2 * KH : MT, cb], AluOpType.add
    )
    nc.scalar.activation(cht, cht, AF.Tanh)
    dt_ = sb.tile([128, KH, BL], F32, tag="dt%d" % lay.li)
    nc.vector.tensor_tensor(dt_, cht, h, AluOpType.subtract)
    zd = sb.tile([128, KH, BL], F32, tag="zd%d" % lay.li)
    nc.vector.tensor_tensor(zd, zrt[:, 0:KH], dt_, AluOpType.mult)
    hn = sb.tile([128, KH, BL], F32R, tag="h%d" % lay.li)
    nc.vector.tensor_tensor(hn, h, zd, AluOpType.add)

    if h0ch is not None:
        nc.vector.tensor_copy(h0ch[:, :, cb], hn)
    return hn


def _contrib_chunk(cx, wx, kx, rhs_tile, bias, pa, capool, tagname, after=()):
    """Dense x-contribution matmuls for one chunk. pa: static (128, 2, NB)
    PSUM tile, halves alternate by m so the m+1 matmuls overlap the m copy.
    `after`: instructions the first matmuls must be ordered behind (the DMA
    absorbers) so those carry the DMA waits instead of the matmuls."""
    nc = cx.nc
    ca = capool.tile([128, MT, NB], F32, tag=tagname)
    for m in range(MT):
        half = pa[:, m % 2]
        for k in range(kx):
            mm = nc.tensor.matmul(
                half,
                wx[:, k, 128 * m : 128 * (m + 1)],
                rhs_tile[:, k],
                start=(k == 0),
                stop=(k == kx - 1),
            )
            if m == 0:
                for a in after:
                    add_dep_helper(mm.ins, a.ins, sync=False, reason="xt absorber first")
        nc.scalar.activation(ca[:, m], half, AF.Identity, bias=bias[:, m : m + 1])
    return ca


def _split_multi_waits(nc):
    """Walrus encodes at most ~2 sync slots per 64-byte compute instruction
    (1 wait + 1 update in practice); Tile's sem assignment happily emits 2-3
    waits. Hoist surplus waits onto an inserted same-engine Drain, which NX
    ucode handles with an unbounded wait list (the kernel-tail drain carries
    11). The drain sits immediately before the offender in its engine queue."""
    import bass_rust

    fn = nc.m.functions[0]
    n = 0
    skip = ("InstEventSemaphore",)
    for b in fn.blocks:
        insts = b.instructions
        newlist = []
        for inst in insts:
            si = getattr(inst, "sync_info", None)
            if (
                si is not None
                and len(si.on_wait) > 1
                and type(inst).__name__ not in skip
            ):
                for w in si.on_wait[:-1]:
                    d = bass_rust.InstDrain(name="waitsplit_%d" % n)
                    d.engine = inst.engine
                    d.sync_info = bass_rust.SyncInfo(on_wait=[w], on_update=[])
                    nc.inst_map[d.name] = d
                    newlist.append(d)
                    n += 1
                inst.sync_info = bass_rust.SyncInfo(
                    on_wait=[si.on_wait[-1]], on_update=list(si.on_update)
                )
            newlist.append(inst)
        insts[:] = newlist
    return n


def build_program(seq_len=S, repeat=1):
    nchunks = seq_len // CH
    nc = bass.Bass()

    xt_d = nc.declare_dram_parameter("xt", [DIN, seq_len * BL], F32R, isOutput=False)
    wzr0_d = nc.declare_dram_parameter("wzr0h", [KH, 128, NZR], F32R, isOutput=False)
    wc0_d = nc.declare_dram_parameter("wc0h", [KH, 128, DH], F32R, isOutput=False)
    wzr1_d = nc.declare_dram_parameter("wzr1h", [KH, 128, NZR], F32R, isOutput=False)
    wc1_d = nc.declare_dram_parameter("wc1h", [KH, 128, DH], F32R, isOutput=False)
    wx0_d = nc.declare_dram_parameter("wx0", [2, 128, NOUT], F32R, isOutput=False)
    w1x_d = nc.declare_dram_parameter("w1x", [KH, 128, NOUT], F32R, isOutput=False)
    b0_d = nc.declare_dram_parameter("b0", [128, MT], F32, isOutput=False)
    b1_d = nc.declare_dram_parameter("b1", [128, MT], F32, isOutput=False)
    id_d = nc.declare_dram_parameter("ident", [128, 128], F32, isOutput=False)
    z_d = nc.declare_dram_parameter("zeros", [128, KH * BL], F32R, isOutput=False)
    hn_d = nc.declare_dram_parameter("hn", [2, BL, DH], F32, isOutput=True)

    cx = Ctx()
    cx.nc = nc

    with tile.TileContext(nc) as tc:
        with (
            tc.tile_pool(name="const", bufs=1) as const,
            tc.tile_pool(name="xc", bufs=2) as xpool,
            tc.tile_pool(name="ca0", bufs=2) as ca0pool,
            tc.tile_pool(name="ca1", bufs=2) as ca1pool,
            tc.tile_pool(name="h0ch", bufs=2) as h0chpool,
            tc.tile_pool(name="sb", bufs=2) as sb,
            tc.tile_pool(name="ps", bufs=1, space="PSUM") as ps,
        ):
            cx.sb = sb

            # resident constants
            wzr0 = const.tile([128, KH, NZR], F32R)
            wc0 = const.tile([128, KH, DH], F32R)
            wzr1 = const.tile([128, KH, NZR], F32R)
            wc1 = const.tile([128, KH, DH], F32R)
            wx0 = const.tile([128, 2, NOUT], F32R)
            w1x = const.tile([128, KH, NOUT], F32R)
            b0 = const.tile([128, MT], F32)
            b1 = const.tile([128, MT], F32)
            ident = const.tile([128, 128], F32)
            cx.ident = ident

            # static PSUM: exactly 8 banks
            lay0, lay1 = Ctx(), Ctx()
            lay0.li, lay1.li = 0, 1
            lay0.wzr, lay0.wc = wzr0, wc0
            lay1.wzr, lay1.wc = wzr1, wc1
            lay0.pzc = ps.tile([BL, DH], F32, tag="pzc0")
            lay0.pr = ps.tile([BL, DH], F32, tag="pr0")
            lay0.pt = ps.tile([128, 16, BL], F32, tag="pt0")
            lay1.pzc = ps.tile([BL, DH], F32, tag="pzc1")
            lay1.pr = ps.tile([BL, DH], F32, tag="pr1")
            lay1.pt = ps.tile([128, 16, BL], F32, tag="pt1")
            paA = ps.tile([128, 2, NB], F32, tag="paA")
            paC = ps.tile([128, 2, NB], F32, tag="paC")

            # Preload everything, then have the PE "observe" each DMA queue via
            # a throwaway transpose, so no real matmul ever needs a DMA wait on
            # top of its producer wait (self-loading f32r matmuls encode only
            # ONE semaphore wait).
            absorb = []
            nc.sync.dma_start(out=ident, in_=id_d[:])
            absorb.append(ident[:, 0:128])
            for k in range(KH):
                nc.sync.dma_start(out=wzr0[:, k], in_=wzr0_d[k])
                absorb.append(wzr0[:, k, 0:128].bitcast(F32))
                nc.sync.dma_start(out=wc0[:, k], in_=wc0_d[k])
                absorb.append(wc0[:, k, 0:128].bitcast(F32))
                nc.sync.dma_start(out=wzr1[:, k], in_=wzr1_d[k])
                absorb.append(wzr1[:, k, 0:128].bitcast(F32))
                nc.sync.dma_start(out=wc1[:, k], in_=wc1_d[k])
                absorb.append(wc1[:, k, 0:128].bitcast(F32))
                nc.sync.dma_start(out=w1x[:, k], in_=w1x_d[k])
                absorb.append(w1x[:, k, 0:128].bitcast(F32))
            for k in range(2):
                nc.sync.dma_start(out=wx0[:, k], in_=wx0_d[k])
                absorb.append(wx0[:, k, 0:128].bitcast(F32))
            nc.sync.dma_start(out=b0, in_=b0_d[:])
            absorb.append(b0)
            nc.sync.dma_start(out=b1, in_=b1_d[:])
            absorb.append(b1)
            for src in absorb:
                nc.tensor.transpose(
                    paA[: src.shape[-1], 0, : src.shape[0]], src, ident
                )

            for _rep in range(repeat):
              h0 = sb.tile([128, KH, BL], F32R, tag="h0")
              h1 = sb.tile([128, KH, BL], F32R, tag="h1")
              nc.sync.dma_start(out=h0, in_=z_d[:])
              nc.sync.dma_start(out=h1, in_=z_d[:])

              for c in range(nchunks):
                # phase A: layer-0 x contributions for chunk c
                # Stage x through an ACT copy: the A matmuls' RAW (copy) and
                # WAR (previous chunk's contrib activation) deps then share
                # the ACT semaphore -> a single wait on the f32r matmul.
                xtr = xpool.tile([128, 2, NB], F32R, tag="xtr")
                xt = xpool.tile([128, 2, NB], F32R, tag="xt")
                for k in range(2):
                    nc.sync.dma_start(
                        out=xtr[:, k], in_=xt_d[128 * k : 128 * (k + 1), NB * c : NB * (c + 1)]
                    )
                    nc.scalar.activation(xt[:, k], xtr[:, k].bitcast(F32), AF.Copy)
                lay0.ca = _contrib_chunk(cx, wx0, 2, xt, b0, paA, ca0pool, "ca0")

                # phase B: layer-0 recurrence over the chunk
                h0ch = h0chpool.tile([128, KH, NB], F32R, tag="h0ch")
                for s in range(CH):
                    h0 = _recurrence_step(cx, lay0, h0, s, h0ch=h0ch)

                # phase C: layer-1 contributions from layer-0 outputs
                lay1.ca = _contrib_chunk(cx, w1x, KH, h0ch, b1, paC, ca1pool, "ca1")

                # phase D: layer-1 recurrence (overlaps next chunk's A/B)
                for s in range(CH):
                    h1 = _recurrence_step(cx, lay1, h1, s, h0ch=None)

            # write back final hidden states, untransposed, via the pt tiles
            for li, h, lay in ((0, h0, lay0), (1, h1, lay1)):
                pf = lay.pt
                for j in range(KH):
                    nc.tensor.transpose(
                        pf[:BL, 4 * j : 4 * (j + 1)], h[:, j].bitcast(F32), ident
                    )
                hf = sb.tile([BL, 16, BL], F32, tag="hf")
                nc.scalar.activation(hf, pf[:BL], AF.Copy)
                nc.sync.dma_start(out=hn_d[li], in_=hf)

    _split_multi_waits(nc)
    return nc


_NC_CACHE = {}


def _get_program(seq_len=S):
    if seq_len not in _NC_CACHE:
        _NC_CACHE[seq_len] = build_program(seq_len)
    return _NC_CACHE[seq_len]


def make_in_maps(x, Wz0, bz0, Wc0, bc0, Wz1, bz1, Wc1, bc1, ncores=NCORES):
    f = lambda a: np.ascontiguousarray(np.asarray(a), dtype=np.float32)
    x = f(x)
    Wz0, bz0, Wc0, bc0 = f(Wz0), f(bz0), f(Wc0), f(bc0)
    Wz1, bz1, Wc1, bc1 = f(Wz1), f(bz1), f(Wc1), f(bc1)
    seq_len = x.shape[1]

    wzr0h = np.ascontiguousarray(Wz0[:DH, :NZR].reshape(KH, 128, NZR))
    wc0h = np.ascontiguousarray(Wc0[:DH].reshape(KH, 128, DH))
    wzr1h = np.ascontiguousarray(Wz1[:DH, :NZR].reshape(KH, 128, NZR))
    wc1h = np.ascontiguousarray(Wc1[:DH].reshape(KH, 128, DH))
    wx0 = np.ascontiguousarray(
        np.concatenate([Wz0[DH : DH + DIN, :NZR], Wc0[DH : DH + DIN]], axis=1).reshape(
            2, 128, NOUT
        )
    )
    w1x = np.ascontiguousarray(
        np.concatenate([Wz1[DH : 2 * DH, :NZR], Wc1[DH : 2 * DH]], axis=1).reshape(
            KH, 128, NOUT
        )
    )
    b0 = np.ascontiguousarray(np.concatenate([bz0[:NZR], bc0]).reshape(MT, 128).T)
    b1 = np.ascontiguousarray(np.concatenate([bz1[:NZR], bc1]).reshape(MT, 128).T)

    in_maps = []
    nb = x.shape[0] // ncores
    for ci in range(ncores):
        xc = x[nb * ci : nb * (ci + 1)]  # (BL, S, DIN)
        xt = np.ascontiguousarray(xc.transpose(2, 1, 0)).reshape(DIN, seq_len * nb)
        in_maps.append(
            dict(
                xt=xt, wzr0h=wzr0h, wc0h=wc0h, wzr1h=wzr1h, wc1h=wc1h,
                wx0=wx0, w1x=w1x, b0=b0, b1=b1,
                ident=np.eye(128, dtype=np.float32),
                zeros=np.zeros((128, KH * BL), dtype=np.float32),
            )
        )
    return in_maps


def kernel(x, Wz0, bz0, Wc0, bc0, Wz1, bz1, Wc1, bc1):
    nc = _get_program(S)
    in_maps = make_in_maps(x, Wz0, bz0, Wc0, bc0, Wz1, bz1, Wc1, bc1)
    res = run_bass_kernel_spmd(nc, in_maps, core_ids=list(range(NCORES)))
    hn = np.concatenate([r["hn"] for r in res.results], axis=1)  # (2, B, DH)
    return hn[-1], hn


# revision 38
# speedup vs baseline: 1.0299x; 1.0299x over previous
"""Trainium2 Bass kernel for a 2-layer custom GRU (batch=256, seq=512,
d_in=256, d_h=512), data-parallel over batch across 8 NeuronCores.

Strategy
--------
- combined = [h, x]; the reference only uses gate chunks z, r (cols 0:1024
  of Wz) -> the third chunk of Wz is dead weight and is trimmed.
- Per layer, weights split into h-part (rows 0:512) and x-part (rows 512:).
  x-contributions (incl. bias) are precomputed chunk-wise as dense matmuls
  (layer 0 from x, layer 1 from layer-0's output sequence), so the serial
  recurrence only does K=512 matmuls per gate.
- Recurrent state h is kept transposed (dims on partitions, batch on free)
  so it is directly the stationary lhsT of the gate matmuls and elementwise
  ops run on full 128 partitions. Gate matmul outputs (batch x dims in
  PSUM) are copied to SBUF and PE-transposed back.
- All matmuls run in float32r (fp32 storage, 1 cycle/column at N>=256).
- Chunks of 8 timesteps pipeline: layer-1 recurrence of chunk c overlaps
  layer-0 recurrence of chunk c+1 (independent dependency chains).
- PSUM tiles are STATIC (one allocation, no pool rotation): the walrus
  encoding of the self-loading f32/f32r matmul holds only ONE semaphore
  wait, so matmuls cannot carry pool-slot release waits on top of their
  producer wait. With static tiles + the emission order below, every PE
  instruction needs at most one wait (WAR hazards are transitively covered
  through same-engine semaphore ticks).
"""

import os
import sys

for _p in ("/opt/trn_rl_repo", "/root/.axon_site/_ro/trn_rl_repo"):
    if os.path.isdir(_p) and _p not in sys.path:
        sys.path.insert(0, _p)

import numpy as np

import concourse.bass as bass
import concourse.mybir as mybir
import concourse.tile as tile
from concourse.alu_op_type import AluOpType
from concourse.bass_utils import run_bass_kernel_spmd
from concourse.tile import add_dep_helper

F32 = mybir.dt.float32
F32R = mybir.dt.float32r
AF = mybir.ActivationFunctionType

B, S, DIN, DH = 256, 512, 256, 512
NCORES = 8
BL = B // NCORES          # 32 batch rows per core
CH = 8                    # timesteps per chunk
NB = CH * BL              # 256 step-batch columns per chunk
NZR = 2 * DH              # 1024: trimmed z|r gate width
NOUT = 3 * DH             # 1536: z|r|c combined output width
KH = DH // 128            # 4 k-tiles for the h contraction
MT = NOUT // 128          # 12 m-tiles of precomputed contributions


class Ctx:
    pass


def _recurrence_step(cx, lay, h, s, h0ch=None):
    """One GRU cell step. h: (128, KH, BL) f32r transposed state.
    lay: per-layer statics (wzr, wc, pzc, pr, pt, ca). Returns new state."""
    nc = cx.nc
    sb = cx.sb
    ident = cx.ident
    wzr, wc, pzc, pr, pt, ca = lay.wzr, lay.wc, lay.pzc, lay.pr, lay.pt, lay.ca
    cb = slice(BL * s, BL * s + BL)

    # gate pre-activations: r first (its WAR+RAW deps are both DVE ticks)
    for k in range(KH):
        nc.tensor.matmul(pr, h[:, k], wzr[:, k, DH:NZR], start=(k == 0), stop=(k == KH - 1))
    for k in range(KH):
        nc.tensor.matmul(pzc, h[:, k], wzr[:, k, 0:DH], start=(k == 0), stop=(k == KH - 1))

    # PSUM -> SBUF (rc on DVE so the r-transposes' deps stay on one engine)
    rc = sb.tile([BL, DH], F32, tag="rc%d" % lay.li)
    zc = sb.tile([BL, DH], F32, tag="zc%d" % lay.li)
    nc.vector.tensor_copy(rc, pr)
    nc.scalar.activation(zc, pzc, AF.Copy)
    for j in range(KH):
        nc.tensor.transpose(pt[:, KH + j], rc[:, 128 * j : 128 * (j + 1)], ident[:BL, :BL])
    for j in range(KH):
        nc.tensor.transpose(pt[:, j], zc[:, 128 * j : 128 * (j + 1)], ident[:BL, :BL])

    # zr = sigmoid(pre + x_contrib)
    zrt = sb.tile([128, 2 * KH, BL], F32, tag="zrt%d" % lay.li)
    nc.vector.tensor_tensor(zrt, pt[:, 0 : 2 * KH], ca[:, 0 : 2 * KH, cb], AluOpType.add)
    nc.scalar.activation(zrt, zrt, AF.Sigmoid)

    # candidate: (r*h) @ Wc_h, reusing the z bank (WAR on zc copy is covered)
    rh = sb.tile([128, KH, BL], F32R, tag="rh%d" % lay.li)
    nc.vector.tensor_tensor(rh, zrt[:, KH : 2 * KH], h, AluOpType.mult)
    for k in range(KH):
        nc.tensor.matmul(pzc, rh[:, k], wc[:, k], start=(k == 0), stop=(k == KH - 1))
    cc = sb.tile([BL, DH], F32, tag="cc%d" % lay.li)
    nc.scalar.activation(cc, pzc, AF.Copy)
    for j in range(KH):
        nc.tensor.transpose(pt[:, 2 * KH + j], cc[:, 128 * j : 128 * (j + 1)], ident[:BL, :BL])

    # h_tilde = tanh(pre + x_contrib); h' = h + z * (h_tilde - h)
    cht = sb.tile([128, KH, BL], F32, tag="cht%d" % lay.li)
    nc.vector.tensor_tensor(
        cht, pt[:, 2 * KH : 3 * KH], ca[:, 2 * KH : MT, cb], AluOpType.add
    )
    nc.scalar.activation(cht, cht, AF.Tanh)
    dt_ = sb.tile([128, KH, BL], F32, tag="dt%d" % lay.li)
    nc.vector.tensor_tensor(dt_, cht, h, AluOpType.subtract)
    zd = sb.tile([128, KH, BL], F32, tag="zd%d" % lay.li)
    nc.vector.tensor_tensor(zd, zrt[:, 0:KH], dt_, AluOpType.mult)
    hn = sb.tile([128, KH, BL], F32R, tag="h%d" % lay.li)
    nc.vector.tensor_tensor(hn, h, zd, AluOpType.add)

    if h0ch is not None:
        nc.vector.tensor_copy(h0ch[:, :, cb], hn)
    return hn


def _contrib_chunk(cx, wx, kx, rhs_tile, bias, pa, capool, tagname, after=()):
    """Dense x-contribution matmuls for one chunk. pa: static (128, 2, NB)
    PSUM tile, halves alternate by m so the m+1 matmuls overlap the m copy.
    `after`: instructions the first matmuls must be ordered behind (the DMA
    absorbers) so those carry the DMA waits instead of the matmuls."""
    nc = cx.nc
    ca = capool.tile([128, MT, NB], F32, tag=tagname)
    for m in range(MT):
        half = pa[:, m % 2]
        for k in range(kx):
            mm = nc.tensor.matmul(
                half,
                wx[:, k, 128 * m : 128 * (m + 1)],
                rhs_tile[:, k],
                start=(k == 0),
                stop=(k == kx - 1),
            )
            if m == 0:
                for a in after:
                    add_dep_helper(mm.ins, a.ins, sync=False, reason="xt absorber first")
        nc.scalar.activation(ca[:, m], half, AF.Identity, bias=bias[:, m : m + 1])
    return ca


def _split_multi_waits(nc):
    """Walrus encodes at most ~2 sync slots per 64-byte compute instruction
    (1 wait + 1 update in practice); Tile's sem assignment happily emits 2-3
    waits. Hoist surplus waits onto an inserted same-engine Drain, which NX
    ucode handles with an unbounded wait list (the kernel-tail drain carries
    11). The drain sits immediately before the offender in its engine queue."""
    import bass_rust

    fn = nc.m.functions[0]
    n = 0
    skip = ("InstEventSemaphore",)
    own_sem = {
        "EngineType.PE": "PE_",
        "EngineType.Activation": "Activation_",
        "EngineType.DVE": "DVE_",
        "EngineType.Pool": "Pool_",
        "EngineType.SP": "SP_",
    }
    for b in fn.blocks:
        insts = b.instructions
        newlist = []
        for inst in insts:
            si = getattr(inst, "sync_info", None)
            if si is not None and len(si.on_wait) > 1 and type(inst).__name__ not in skip:
                # Own-engine waits are redundant: these queues dispatch and
                # complete strictly in order (DVE/ACT drain between ops; PE
                # matmul writes are pc-monotone), so a wait on the engine's
                # own completion semaphore is always already satisfied.
                pre = own_sem.get(str(inst.engine), "\0")
                kept = [w for w in si.on_wait if not w.ant_name.startswith(pre)]
                if len(kept) != len(si.on_wait):
                    si = bass_rust.SyncInfo(on_wait=kept, on_update=list(si.on_update))
                    inst.sync_info = si
            if (
                si is not None
                and len(si.on_wait) > 1
                and type(inst).__name__ not in skip
            ):
                for w in si.on_wait[:-1]:
                    d = bass_rust.InstDrain(name="waitsplit_%d" % n)
                    d.engine = inst.engine
                    d.sync_info = bass_rust.SyncInfo(on_wait=[w], on_update=[])
                    nc.inst_map[d.name] = d
                    newlist.append(d)
                    n += 1
                inst.sync_info = bass_rust.SyncInfo(
                    on_wait=[si.on_wait[-1]], on_update=list(si.on_update)
                )
            newlist.append(inst)
        insts[:] = newlist
    return n


def build_program(seq_len=S, repeat=1):
    nchunks = seq_len // CH
    nc = bass.Bass()

    xt_d = nc.declare_dram_parameter("xt", [DIN, seq_len * BL], F32R, isOutput=False)
    wzr0_d = nc.declare_dram_parameter("wzr0h", [KH, 128, NZR], F32R, isOutput=False)
    wc0_d = nc.declare_dram_parameter("wc0h", [KH, 128, DH], F32R, isOutput=False)
    wzr1_d = nc.declare_dram_parameter("wzr1h", [KH, 128, NZR], F32R, isOutput=False)
    wc1_d = nc.declare_dram_parameter("wc1h", [KH, 128, DH], F32R, isOutput=False)
    wx0_d = nc.declare_dram_parameter("wx0", [2, 128, NOUT], F32R, isOutput=False)
    w1x_d = nc.declare_dram_parameter("w1x", [KH, 128, NOUT], F32R, isOutput=False)
    b0_d = nc.declare_dram_parameter("b0", [128, MT], F32, isOutput=False)
    b1_d = nc.declare_dram_parameter("b1", [128, MT], F32, isOutput=False)
    id_d = nc.declare_dram_parameter("ident", [128, 128], F32, isOutput=False)
    z_d = nc.declare_dram_parameter("zeros", [128, KH * BL], F32R, isOutput=False)
    hn_d = nc.declare_dram_parameter("hn", [2, BL, DH], F32, isOutput=True)

    cx = Ctx()
    cx.nc = nc

    with tile.TileContext(nc) as tc:
        with (
            tc.tile_pool(name="const", bufs=1) as const,
            tc.tile_pool(name="xc", bufs=2) as xpool,
            tc.tile_pool(name="ca0", bufs=2) as ca0pool,
            tc.tile_pool(name="ca1", bufs=2) as ca1pool,
            tc.tile_pool(name="h0ch", bufs=2) as h0chpool,
            tc.tile_pool(name="sb", bufs=2) as sb,
            tc.tile_pool(name="ps", bufs=1, space="PSUM") as ps,
        ):
            cx.sb = sb

            # resident constants
            wzr0 = const.tile([128, KH, NZR], F32R)
            wc0 = const.tile([128, KH, DH], F32R)
            wzr1 = const.tile([128, KH, NZR], F32R)
            wc1 = const.tile([128, KH, DH], F32R)
            wx0 = const.tile([128, 2, NOUT], F32R)
            w1x = const.tile([128, KH, NOUT], F32R)
            b0 = const.tile([128, MT], F32)
            b1 = const.tile([128, MT], F32)
            ident = const.tile([128, 128], F32)
            cx.ident = ident

            # static PSUM: exactly 8 banks
            lay0, lay1 = Ctx(), Ctx()
            lay0.li, lay1.li = 0, 1
            lay0.wzr, lay0.wc = wzr0, wc0
            lay1.wzr, lay1.wc = wzr1, wc1
            lay0.pzc = ps.tile([BL, DH], F32, tag="pzc0")
            lay0.pr = ps.tile([BL, DH], F32, tag="pr0")
            lay0.pt = ps.tile([128, 16, BL], F32, tag="pt0")
            lay1.pzc = ps.tile([BL, DH], F32, tag="pzc1")
            lay1.pr = ps.tile([BL, DH], F32, tag="pr1")
            lay1.pt = ps.tile([128, 16, BL], F32, tag="pt1")
            paA = ps.tile([128, 2, NB], F32, tag="paA")
            paC = ps.tile([128, 2, NB], F32, tag="paC")

            # Preload everything, then have the PE "observe" each DMA queue via
            # a throwaway transpose, so no real matmul ever needs a DMA wait on
            # top of its producer wait (self-loading f32r matmuls encode only
            # ONE semaphore wait).
            absorb = []
            nc.sync.dma_start(out=ident, in_=id_d[:])
            absorb.append(ident[:, 0:128])
            for k in range(KH):
                nc.sync.dma_start(out=wzr0[:, k], in_=wzr0_d[k])
                absorb.append(wzr0[:, k, 0:128].bitcast(F32))
                nc.sync.dma_start(out=wc0[:, k], in_=wc0_d[k])
                absorb.append(wc0[:, k, 0:128].bitcast(F32))
                nc.sync.dma_start(out=wzr1[:, k], in_=wzr1_d[k])
                absorb.append(wzr1[:, k, 0:128].bitcast(F32))
                nc.sync.dma_start(out=wc1[:, k], in_=wc1_d[k])
                absorb.append(wc1[:, k, 0:128].bitcast(F32))
                nc.sync.dma_start(out=w1x[:, k], in_=w1x_d[k])
                absorb.append(w1x[:, k, 0:128].bitcast(F32))
            for k in range(2):
                nc.sync.dma_start(out=wx0[:, k], in_=wx0_d[k])
                absorb.append(wx0[:, k, 0:128].bitcast(F32))
            nc.sync.dma_start(out=b0, in_=b0_d[:])
            absorb.append(b0)
            nc.sync.dma_start(out=b1, in_=b1_d[:])
            absorb.append(b1)
            for src in absorb:
                nc.tensor.transpose(
                    paA[: src.shape[-1], 0, : src.shape[0]], src, ident
                )

            for _rep in range(repeat):
              h0 = sb.tile([128, KH, BL], F32R, tag="h0")
              h1 = sb.tile([128, KH, BL], F32R, tag="h1")
              nc.sync.dma_start(out=h0, in_=z_d[:])
              nc.sync.dma_start(out=h1, in_=z_d[:])

              for c in range(nchunks):
                # phase A: layer-0 x contributions for chunk c
                # Stage x through an ACT copy: the A matmuls' RAW (copy) and
                # WAR (previous chunk's contrib activation) deps then share
                # the ACT semaphore -> a single wait on the f32r matmul.
                xtr = xpool.tile([128, 2, NB], F32R, tag="xtr")
                xt = xpool.tile([128, 2, NB], F32R, tag="xt")
                for k in range(2):
                    nc.sync.dma_start(
                        out=xtr[:, k], in_=xt_d[128 * k : 128 * (k + 1), NB * c : NB * (c + 1)]
                    )
                    nc.scalar.activation(xt[:, k], xtr[:, k].bitcast(F32), AF.Copy)
                lay0.ca = _contrib_chunk(cx, wx0, 2, xt, b0, paA, ca0pool, "ca0")

                # phase B: layer-0 recurrence over the chunk
                h0ch = h0chpool.tile([128, KH, NB], F32R, tag="h0ch")
                for s in range(CH):
                    h0 = _recurrence_step(cx, lay0, h0, s, h0ch=h0ch)

                # phase C: layer-1 contributions from layer-0 outputs
                lay1.ca = _contrib_chunk(cx, w1x, KH, h0ch, b1, paC, ca1pool, "ca1")

                # phase D: layer-1 recurrence (overlaps next chunk's A/B)
                for s in range(CH):
                    h1 = _recurrence_step(cx, lay1, h1, s, h0ch=None)

            # write back final hidden states, untransposed, via the pt tiles
            for li, h, lay in ((0, h0, lay0), (1, h1, lay1)):
                pf = lay.pt
                for j in range(KH):
                    nc.tensor.transpose(
                        pf[:BL, 4 * j : 4 * (j + 1)], h[:, j].bitcast(F32), ident
                    )
                hf = sb.tile([BL, 16, BL], F32, tag="hf")
                nc.scalar.activation(hf, pf[:BL], AF.Copy)
                nc.sync.dma_start(out=hn_d[li], in_=hf)

    _split_multi_waits(nc)
    return nc


_NC_CACHE = {}


def _get_program(seq_len=S):
    if seq_len not in _NC_CACHE:
        _NC_CACHE[seq_len] = build_program(seq_len)
    return _NC_CACHE[seq_len]


def make_in_maps(x, Wz0, bz0, Wc0, bc0, Wz1, bz1, Wc1, bc1, ncores=NCORES):
    f = lambda a: np.ascontiguousarray(np.asarray(a), dtype=np.float32)
    x = f(x)
    Wz0, bz0, Wc0, bc0 = f(Wz0), f(bz0), f(Wc0), f(bc0)
    Wz1, bz1, Wc1, bc1 = f(Wz1), f(bz1), f(Wc1), f(bc1)
    seq_len = x.shape[1]

    wzr0h = np.ascontiguousarray(Wz0[:DH, :NZR].reshape(KH, 128, NZR))
    wc0h = np.ascontiguousarray(Wc0[:DH].reshape(KH, 128, DH))
    wzr1h = np.ascontiguousarray(Wz1[:DH, :NZR].reshape(KH, 128, NZR))
    wc1h = np.ascontiguousarray(Wc1[:DH].reshape(KH, 128, DH))
    wx0 = np.ascontiguousarray(
        np.concatenate([Wz0[DH : DH + DIN, :NZR], Wc0[DH : DH + DIN]], axis=1).reshape(
            2, 128, NOUT
        )
    )
    w1x = np.ascontiguousarray(
        np.concatenate([Wz1[DH : 2 * DH, :NZR], Wc1[DH : 2 * DH]], axis=1).reshape(
            KH, 128, NOUT
        )
    )
    b0 = np.ascontiguousarray(np.concatenate([bz0[:NZR], bc0]).reshape(MT, 128).T)
    b1 = np.ascontiguousarray(np.concatenate([bz1[:NZR], bc1]).reshape(MT, 128).T)

    in_maps = []
    nb = x.shape[0] // ncores
    for ci in range(ncores):
        xc = x[nb * ci : nb * (ci + 1)]  # (BL, S, DIN)
        xt = np.ascontiguousarray(xc.transpose(2, 1, 0)).reshape(DIN, seq_len * nb)
        in_maps.append(
            dict(
                xt=xt, wzr0h=wzr0h, wc0h=wc0h, wzr1h=wzr1h, wc1h=wc1h,
                wx0=wx0, w1x=w1x, b0=b0, b1=b1,
                ident=np.eye(128, dtype=np.float32),
                zeros=np.zeros((128, KH * BL), dtype=np.float32),
            )
        )
    return in_maps


def kernel(x, Wz0, bz0, Wc0, bc0, Wz1, bz1, Wc1, bc1):
    nc = _get_program(S)
    in_maps = make_in_maps(x, Wz0, bz0, Wc0, bc0, Wz1, bz1, Wc1, bc1)
    res = run_bass_kernel_spmd(nc, in_maps, core_ids=list(range(NCORES)))
    hn = np.concatenate([r["hn"] for r in res.results], axis=1)  # (2, B, DH)
    return hn[-1], hn


# revision 48
# speedup vs baseline: 1.5060x; 1.4623x over previous
"""Trainium2 Bass kernel for a 2-layer custom GRU (batch=256, seq=512,
d_in=256, d_h=512), data-parallel over batch across 8 NeuronCores.

The execution platform steps instructions at ~30-50us each (engines
serialize, cores mostly parallel), so the design minimizes INSTRUCTION
COUNT per core:

- The reference only uses gate chunks z, r (cols 0:1024 of Wz) -> the
  third chunk of Wz is trimmed away.
- Per layer, weights split into h-part (rows 0:512) and x-part. The
  x-contributions (incl. bias) are precomputed chunk-wise as dense
  matmuls in (step-batch, out-dims) layout: layer 0 from x, layer 1
  from layer-0's output chunk.
- Recurrent elementwise runs in (batch, dims) layout: one TT+sigmoid for
  both z|r (psum pair read as (32,1024)), one TT+tanh for the candidate,
  3 TTs for the state update.
- The two tensors that feed matmuls as stationary lhsT (r*h and h') are
  PE-transposed (4 transposes + 1 copy each); the h' copy lands directly
  in the chunk buffer that phase C consumes.
- All matmuls in float32r (fp32 storage, PE fast path). Producers of
  f32r matmul inputs declare f32r outputs (walrus rounding rule).
- A post-pass splits multi-wait instructions (walrus encodes only one
  semaphore wait per compute instruction) by hoisting surplus waits onto
  inserted same-engine Drains, after deleting redundant own-engine waits.
"""

import os
import sys

for _p in ("/opt/trn_rl_repo", "/root/.axon_site/_ro/trn_rl_repo"):
    if os.path.isdir(_p) and _p not in sys.path:
        sys.path.insert(0, _p)

import numpy as np

import concourse.bass as bass
import concourse.mybir as mybir
import concourse.tile as tile
from concourse.alu_op_type import AluOpType
from concourse.bass_utils import run_bass_kernel_spmd

F32 = mybir.dt.float32
F32R = mybir.dt.float32r
AF = mybir.ActivationFunctionType

B, S, DIN, DH = 256, 512, 256, 512
NCORES = 8
BL = B // NCORES          # 32 batch rows per core
CH = 8                    # timesteps per chunk
NB = CH * BL              # 256 step-batch rows per chunk (2 m-tiles of 128)
NZR = 2 * DH              # 1024: trimmed z|r gate width
NOUT = 3 * DH             # 1536: z|r|c combined output width
KH = DH // 128            # 4 k-tiles for the h contraction


class Ctx:
    pass


def _split_multi_waits(nc):
    """Walrus encodes at most one semaphore wait per compute instruction.
    First delete redundant own-engine waits (queues are strict in-order with
    inter-op drains), then hoist surplus waits onto inserted same-engine
    Drains (one wait each)."""
    import bass_rust

    fn = nc.m.functions[0]
    n = 0
    skip = ("InstEventSemaphore",)
    own_sem = {
        "EngineType.PE": "PE_",
        "EngineType.Activation": "Activation_",
        "EngineType.DVE": "DVE_",
        "EngineType.Pool": "Pool_",
        "EngineType.SP": "SP_",
    }
    for b in fn.blocks:
        insts = b.instructions
        newlist = []
        for inst in insts:
            si = getattr(inst, "sync_info", None)
            if si is not None and len(si.on_wait) > 1 and type(inst).__name__ not in skip:
                pre = own_sem.get(str(inst.engine), "\0")
                kept = [w for w in si.on_wait if not w.ant_name.startswith(pre)]
                if len(kept) != len(si.on_wait):
                    si = bass_rust.SyncInfo(on_wait=kept, on_update=list(si.on_update))
                    inst.sync_info = si
            if (
                si is not None
                and len(si.on_wait) > 1
                and type(inst).__name__ not in skip
            ):
                for w in si.on_wait[:-1]:
                    d = bass_rust.InstDrain(name="waitsplit_%d" % n)
                    d.engine = inst.engine
                    d.sync_info = bass_rust.SyncInfo(on_wait=[w], on_update=[])
                    nc.inst_map[d.name] = d
                    newlist.append(d)
                    n += 1
                inst.sync_info = bass_rust.SyncInfo(
                    on_wait=[si.on_wait[-1]], on_update=list(si.on_update)
                )
            newlist.append(inst)
        insts[:] = newlist
    return n


def _step(cx, lay, hE, hT, s, h0ch=None):
    """One GRU cell step.
    hE: (BL, DH) f32 elementwise state; hT: (128, KH, BL) f32r lhsT state.
    Returns (hE', hT')."""
    nc = cx.nc
    sb = cx.sb
    ident = cx.ident
    li = lay.li
    # per-step contribution slice: (BL, NOUT) rows of the chunk's (128,2,NOUT)
    r0 = BL * (s % 4)
    mi = s // 4
    caz = lay.ca[r0 : r0 + BL, mi, 0:NZR]
    cac = lay.ca[r0 : r0 + BL, mi, NZR:NOUT]

    # z and r gate pre-activations into the psum pair (two banks)
    for k in range(KH):
        nc.tensor.matmul(lay.pzr[:, 0], hT[:, k], lay.wzr[:, k, 0:DH],
                         start=(k == 0), stop=(k == KH - 1))
    for k in range(KH):
        nc.tensor.matmul(lay.pzr[:, 1], hT[:, k], lay.wzr[:, k, DH:NZR],
                         start=(k == 0), stop=(k == KH - 1))

    # zr = sigmoid(pre + contrib) : one TT over both banks + one activation
    zr = sb.tile([BL, 2, DH], F32, tag="zr%d" % li)
    nc.vector.tensor_tensor(zr, lay.pzr, caz.rearrange("b (g d) -> b g d", g=2), AluOpType.add)
    nc.scalar.activation(zr, zr, AF.Sigmoid)

    # rh = r * h, then transpose it for use as lhsT of the candidate matmul
    rh = sb.tile([BL, DH], F32, tag="rh%d" % li)
    nc.vector.tensor_tensor(rh, zr[:, 1], hE, AluOpType.mult)
    for j in range(KH):
        nc.tensor.transpose(cx.pT[:, j], rh[:, 128 * j : 128 * (j + 1)], ident[:BL, :BL])
    rhT = sb.tile([128, KH, BL], F32R, tag="rhT%d" % li)
    nc.vector.tensor_copy(rhT, cx.pT)

    # candidate: tanh((r*h) @ Wc_h + contrib)
    for k in range(KH):
        nc.tensor.matmul(lay.pc, rhT[:, k], lay.wc[:, k], start=(k == 0), stop=(k == KH - 1))
    ht = sb.tile([BL, DH], F32, tag="ht%d" % li)
    nc.vector.tensor_tensor(ht, lay.pc, cac, AluOpType.add)
    nc.scalar.activation(ht, ht, AF.Tanh)

    # h' = h + z*(ht - h)
    d_ = sb.tile([BL, DH], F32, tag="d%d" % li)
    nc.vector.tensor_tensor(d_, ht, hE, AluOpType.subtract)
    zd = sb.tile([BL, DH], F32, tag="zd%d" % li)
    nc.vector.tensor_tensor(zd, zr[:, 0], d_, AluOpType.mult)
    hEn = sb.tile([BL, DH], F32R, tag="hE%d" % li, bufs=2)
    nc.vector.tensor_tensor(hEn, hE, zd, AluOpType.add)

    # transpose h' for the next step's lhsT (and phase C for layer 0)
    for j in range(KH):
        nc.tensor.transpose(
            cx.pT[:, j], hEn[:, 128 * j : 128 * (j + 1)].bitcast(F32), ident[:BL, :BL]
        )
    if h0ch is not None:
        hTn = h0ch[:, :, BL * s : BL * (s + 1)]
    else:
        hTn = sb.tile([128, KH, BL], F32R, tag="hT%d" % li)
    nc.vector.tensor_copy(hTn, cx.pT)
    return hEn, hTn


def _contrib_chunk(cx, wx, kx, lhs, bias, capool, tagname):
    """Dense x-contribution matmuls for one chunk, orientation-1:
    ca[m] (128 step-batch rows, NOUT) = lhs_m @ wx + bias.
    lhs: (128, kx, NB) f32r (step-batch on the free axis, so lhsT blocks are
    (128, 128) slices). Output layout (128, 2, NOUT)."""
    nc = cx.nc
    ca = capool.tile([128, 2, NOUT], F32, tag=tagname)
    for m in range(2):
        for ni in range(3):
            half = cx.pa[:, ni % 2]
            for k in range(kx):
                nc.tensor.matmul(
                    half,
                    lhs[:, k, 128 * m : 128 * (m + 1)],
                    wx[:, k, 512 * ni : 512 * (ni + 1)],
                    start=(k == 0),
                    stop=(k == kx - 1),
                )
            nc.vector.tensor_tensor(
                ca[:, m, 512 * ni : 512 * (ni + 1)],
                half,
                bias[:, 512 * ni : 512 * (ni + 1)],
                AluOpType.add,
            )
    return ca


def build_program(seq_len=S, repeat=1):
    nchunks = seq_len // CH
    nc = bass.Bass()

    xt_d = nc.declare_dram_parameter("xt", [DIN, seq_len * BL], F32R, isOutput=False)
    wzr0_d = nc.declare_dram_parameter("wzr0h", [KH, 128, NZR], F32R, isOutput=False)
    wc0_d = nc.declare_dram_parameter("wc0h", [KH, 128, DH], F32R, isOutput=False)
    wzr1_d = nc.declare_dram_parameter("wzr1h", [KH, 128, NZR], F32R, isOutput=False)
    wc1_d = nc.declare_dram_parameter("wc1h", [KH, 128, DH], F32R, isOutput=False)
    wx0_d = nc.declare_dram_parameter("wx0", [2, 128, NOUT], F32R, isOutput=False)
    w1x_d = nc.declare_dram_parameter("w1x", [KH, 128, NOUT], F32R, isOutput=False)
    b0_d = nc.declare_dram_parameter("b0", [128, NOUT], F32, isOutput=False)
    b1_d = nc.declare_dram_parameter("b1", [128, NOUT], F32, isOutput=False)
    id_d = nc.declare_dram_parameter("ident", [128, 128], F32, isOutput=False)
    z_d = nc.declare_dram_parameter("zeros", [128, DH], F32R, isOutput=False)
    hn_d = nc.declare_dram_parameter("hn", [2, BL, DH], F32, isOutput=True)

    cx = Ctx()
    cx.nc = nc

    with tile.TileContext(nc) as tc:
        with (
            tc.tile_pool(name="const", bufs=1) as const,
            tc.tile_pool(name="xc", bufs=1) as xpool,
            tc.tile_pool(name="ca0", bufs=1) as ca0pool,
            tc.tile_pool(name="ca1", bufs=1) as ca1pool,
            tc.tile_pool(name="h0ch", bufs=1) as h0chpool,
            tc.tile_pool(name="sb", bufs=1) as sb,
            tc.tile_pool(name="ps", bufs=1, space="PSUM") as ps,
        ):
            cx.sb = sb

            wzr0 = const.tile([128, KH, NZR], F32R)
            wc0 = const.tile([128, KH, DH], F32R)
            wzr1 = const.tile([128, KH, NZR], F32R)
            wc1 = const.tile([128, KH, DH], F32R)
            wx0 = const.tile([128, 2, NOUT], F32R)
            w1x = const.tile([128, KH, NOUT], F32R)
            b0 = const.tile([128, NOUT], F32)
            b1 = const.tile([128, NOUT], F32)
            ident = const.tile([128, 128], F32)
            cx.ident = ident

            lay0, lay1 = Ctx(), Ctx()
            lay0.li, lay1.li = 0, 1
            lay0.wzr, lay0.wc = wzr0, wc0
            lay1.wzr, lay1.wc = wzr1, wc1
            # static PSUM: 2+2+1+1+2 = 8 banks
            lay0.pzr = ps.tile([BL, 2, DH], F32, tag="pzr0")
            lay1.pzr = ps.tile([BL, 2, DH], F32, tag="pzr1")
            lay0.pc = ps.tile([BL, DH], F32, tag="pc0")
            lay1.pc = lay0.pc
            cx.pT = ps.tile([128, KH, BL], F32, tag="pT")
            cx.pa = ps.tile([128, 2, DH], F32, tag="pa")

            nc.sync.dma_start(out=ident, in_=id_d[:])
            for k in range(KH):
                nc.sync.dma_start(out=wzr0[:, k], in_=wzr0_d[k])
                nc.sync.dma_start(out=wc0[:, k], in_=wc0_d[k])
                nc.sync.dma_start(out=wzr1[:, k], in_=wzr1_d[k])
                nc.sync.dma_start(out=wc1[:, k], in_=wc1_d[k])
                nc.sync.dma_start(out=w1x[:, k], in_=w1x_d[k])
            for k in range(2):
                nc.sync.dma_start(out=wx0[:, k], in_=wx0_d[k])
            nc.sync.dma_start(out=b0, in_=b0_d[:])
            nc.sync.dma_start(out=b1, in_=b1_d[:])

            for _rep in range(repeat):
                hT0 = sb.tile([128, KH, BL], F32R, tag="hT0i")
                hT1 = sb.tile([128, KH, BL], F32R, tag="hT1i")
                hE0 = sb.tile([BL, DH], F32R, tag="hE0", bufs=2)
                hE1 = sb.tile([BL, DH], F32R, tag="hE1", bufs=2)
                nc.sync.dma_start(out=hT0, in_=z_d[:, 0 : KH * BL])
                nc.sync.dma_start(out=hT1, in_=z_d[:, 0 : KH * BL])
                nc.sync.dma_start(out=hE0, in_=z_d[0:BL, :])
                nc.sync.dma_start(out=hE1, in_=z_d[0:BL, :])

                for c in range(nchunks):
                    # phase A: layer-0 x contributions
                    xt = xpool.tile([128, 2, NB], F32R, tag="xt")
                    for k in range(2):
                        nc.sync.dma_start(
                            out=xt[:, k],
                            in_=xt_d[128 * k : 128 * (k + 1), NB * c : NB * (c + 1)],
                        )
                    lay0.ca = _contrib_chunk(cx, wx0, 2, xt, b0, ca0pool, "ca0")

                    # phase B: layer-0 recurrence; h' transposes land in h0ch
                    h0ch = h0chpool.tile([128, KH, NB], F32R, tag="h0ch")
                    for s in range(CH):
                        hE0, hT0 = _step(cx, lay0, hE0, hT0, s, h0ch=h0ch)

                    # phase C: layer-1 contributions from layer-0 outputs
                    lay1.ca = _contrib_chunk(cx, w1x, KH, h0ch, b1, ca1pool, "ca1")

                    # phase D: layer-1 recurrence
                    for s in range(CH):
                        hE1, hT1 = _step(cx, lay1, hE1, hT1, s, h0ch=None)

            # final hidden states are already in (batch, dims) layout
            for li, hE in ((0, hE0), (1, hE1)):
                nc.sync.dma_start(out=hn_d[li], in_=hE.bitcast(F32))

    _split_multi_waits(nc)
    return nc


_NC_CACHE = {}


def _get_program(seq_len=S, repeat=1):
    key = (seq_len, repeat)
    if key not in _NC_CACHE:
        _NC_CACHE[key] = build_program(seq_len, repeat)
    return _NC_CACHE[key]


def make_in_maps(x, Wz0, bz0, Wc0, bc0, Wz1, bz1, Wc1, bc1, ncores=NCORES):
    f = lambda a: np.ascontiguousarray(np.asarray(a), dtype=np.float32)
    x = f(x)
    Wz0, bz0, Wc0, bc0 = f(Wz0), f(bz0), f(Wc0), f(bc0)
    Wz1, bz1, Wc1, bc1 = f(Wz1), f(bz1), f(Wc1), f(bc1)
    seq_len = x.shape[1]

    wzr0h = np.ascontiguousarray(Wz0[:DH, :NZR].reshape(KH, 128, NZR))
    wc0h = np.ascontiguousarray(Wc0[:DH].reshape(KH, 128, DH))
    wzr1h = np.ascontiguousarray(Wz1[:DH, :NZR].reshape(KH, 128, NZR))
    wc1h = np.ascontiguousarray(Wc1[:DH].reshape(KH, 128, DH))
    wx0 = np.ascontiguousarray(
        np.concatenate([Wz0[DH : DH + DIN, :NZR], Wc0[DH : DH + DIN]], axis=1).reshape(
            2, 128, NOUT
        )
    )
    w1x = np.ascontiguousarray(
        np.concatenate([Wz1[DH : 2 * DH, :NZR], Wc1[DH : 2 * DH]], axis=1).reshape(
            KH, 128, NOUT
        )
    )
    brow0 = np.concatenate([bz0[:NZR], bc0])
    brow1 = np.concatenate([bz1[:NZR], bc1])
    b0 = np.ascontiguousarray(np.tile(brow0, (128, 1)))
    b1 = np.ascontiguousarray(np.tile(brow1, (128, 1)))

    in_maps = []
    nb = x.shape[0] // ncores
    for ci in range(ncores):
        xc = x[nb * ci : nb * (ci + 1)]  # (BL, S, DIN)
        xt = np.ascontiguousarray(xc.transpose(2, 1, 0)).reshape(DIN, seq_len * nb)
        in_maps.append(
            dict(
                xt=xt, wzr0h=wzr0h, wc0h=wc0h, wzr1h=wzr1h, wc1h=wc1h,
                wx0=wx0, w1x=w1x, b0=b0, b1=b1,
                ident=np.eye(128, dtype=np.float32),
                zeros=np.zeros((128, DH), dtype=np.float32),
            )
        )
    return in_maps


def kernel(x, Wz0, bz0, Wc0, bc0, Wz1, bz1, Wc1, bc1):
    nc = _get_program(S)
    in_maps = make_in_maps(x, Wz0, bz0, Wc0, bc0, Wz1, bz1, Wc1, bc1)
    res = run_bass_kernel_spmd(nc, in_maps, core_ids=list(range(NCORES)))
    hn = np.concatenate([r["hn"] for r in res.results], axis=1)  # (2, B, DH)
    return hn[-1], hn
